# revision 10
# baseline (speedup 1.0000x reference)
"""Trainium2 Bass kernel for nn_DiversityMetric (batched NND diversity metric).

Math (per batch b, X = pred_poses[b] in R^{N x D}, N=2048, D=128):
    sq_dist[i,j] = ||xi||^2 + ||xj||^2 - 2 <xi,xj>, diag = inf
    nnd[i]       = sqrt(min_{j != i} sq_dist[i,j])
    out          = [mean(nnd), std(nnd, ddof=1), cv]   over all B*N points

v3 design (8 cores, 2 batches/core, bf16 matmul path):
  PE per row-strip m (128 rows x 2048 cols of the gram):
    - 4x [K=128, N=512] bf16 matmuls  v_ij = <xi, xj>         (start)
    - 1x identneg matmul: diag -= 1e6 (self-exclusion)
    - 4x row-tiled K=1 matmuls (tile_position=(32c,0), concurrent):
        v_ij += -0.5*sqn_j   -- lhsT = [-0.5] row, rhs = sqn_row chunk c
  Batch setup (borrows one PSUM slot):
    - sqn_row via 4 col-tiled K=128/M=32 matmuls (tile_position=(0,32c))
    - per-point sqn columns via 16 tiny K=128/N=2 matmuls
  Drain, strictly alternating across both PSUM-capable engines:
    'R': DVE tensor_reduce(max) per [128,1024] half -> rmax2[:,col,h]
    'S': ACT softmin: accum = sum_j exp(t*v_ij + bias_i) in one pass,
         bias_i = -t/2*(sqn_i - C).  max_j v_ij ~= (ln acc - bias)/t
         (high-dim NN gaps make exp(t*(v-vmax)) tiny), so
         nnd_i^2 = sqn_i - 2*max = C - (2/t)*ln(acc): host-side.
  PSUM: single pool of 4x [128,1024] slots (8 banks) so PE always has
  two free halves per strip -- avoids PE idle gaps (HAM p-state drops).
  Host computes nnd2/sqrt/mean/std/cv from rmax2, lsum2, sqn (f64).
"""

import numpy as np
from contextlib import ExitStack

import ml_dtypes

import concourse.bass as bass
import concourse.bacc as bacc
import concourse.mybir as mybir
import concourse.tile as tile
from concourse.bass_utils import run_bass_kernel_spmd

F32 = mybir.dt.float32
BF16 = mybir.dt.bfloat16

B, N, D = 16, 2048, 128
NCORES = 8
BPC = B // NCORES          # batches per core
P = 128                    # partitions
MBLK = N // P              # 16 row strips per batch
MMW = 512                  # matmul moving width (1 PSUM bank)
CHUNK = 1024               # xt/sq SBUF chunk width
NEGBIG = -1.0e6

SOFT_T = 2.5               # softmin sharpness
SOFT_C = 160.0             # centering constant (~E[nnd^2]); exactly cancels

OFFS_TILED = True          # row-tiled K=1 offset matmuls (else neghalf K=128)

# Per-half drain assignment: h0 -> DVE exact reduce ('R'), h1 -> ACT softmin
# ('S'), except EXTRA_R strips which send both halves to DVE (load balance).
EXTRA_R = set(range(BPC * MBLK))     # all-DVE experiment
PAT2 = [
    ('R', 'R') if i in EXTRA_R else ('R', 'S')
    for i in range(BPC * MBLK)
]

_CACHE = {}


def build_kernel():
    nc = bacc.Bacc("TRN2", target_bir_lowering=False, debug=False)

    xt_d = nc.dram_tensor("xt", [BPC, P, N], BF16, kind="ExternalInput")
    neghalf_d = nc.dram_tensor("neghalf", [P, P], BF16, kind="ExternalInput")
    ones_d = nc.dram_tensor("ones32", [P, 32], BF16, kind="ExternalInput")
    identneg_d = nc.dram_tensor("identneg", [P, P], BF16, kind="ExternalInput")
    ident_d = nc.dram_tensor("ident", [P, P], BF16, kind="ExternalInput")
    rmax2_d = nc.dram_tensor("rmax2", [P, BPC * MBLK, 2], F32,
                             kind="ExternalOutput")
    lsum2_d = nc.dram_tensor("lsum2", [P, BPC * MBLK, 2], F32,
                             kind="ExternalOutput")
    sqn_d = nc.dram_tensor("sqn", [P, BPC * MBLK], F32, kind="ExternalOutput")

    with tile.TileContext(nc) as tc, ExitStack() as ctx:
        const = ctx.enter_context(tc.tile_pool(name="const", bufs=1))
        xpool = ctx.enter_context(tc.tile_pool(name="x", bufs=1))
        spool = ctx.enter_context(tc.tile_pool(name="s", bufs=1))
        small = ctx.enter_context(tc.tile_pool(name="small", bufs=1))
        rowp = ctx.enter_context(tc.tile_pool(name="rowp", bufs=1))
        junkp = ctx.enter_context(tc.tile_pool(name="junk", bufs=1))
        psum = ctx.enter_context(tc.tile_pool(name="psum", bufs=4, space="PSUM"))

        NCHUNK = N // CHUNK
        xts = {}
        sqs = {}

        def load_chunk(b, c):
            xtile = xpool.tile([P, CHUNK], BF16, tag=f"xt_{b}_{c}")
            nc.sync.dma_start(
                xtile[:], xt_d.ap()[b, :, c * CHUNK:(c + 1) * CHUNK]
            )
            xts[(b, c)] = xtile
            stile = spool.tile([P, CHUNK], BF16, tag=f"sq_{b}_{c}")
            if b == 0 and c == 0:
                nc.scalar.square(stile[:], xtile[:])
            elif b == 0:
                nc.vector.tensor_tensor(
                    stile[:], xtile[:], xtile[:], op=mybir.AluOpType.mult
                )
            else:
                nc.gpsimd.tensor_mul(stile[:], xtile[:], xtile[:])
            sqs[(b, c)] = stile

        load_chunk(0, 0)

        neghalf = const.tile([P, P], BF16)
        nc.scalar.dma_start(neghalf[:], neghalf_d.ap())
        ones32 = const.tile([P, 32], BF16)
        nc.scalar.dma_start(ones32[:], ones_d.ap())
        identneg = const.tile([P, P], BF16)
        nc.scalar.dma_start(identneg[:], identneg_d.ap())
        ident = const.tile([P, P], BF16)
        nc.scalar.dma_start(ident[:], ident_d.ap())

        for b in range(BPC):
            for c in range(NCHUNK):
                if (b, c) != (0, 0):
                    load_chunk(b, c)

        def xcol(b, j0, w):
            c = j0 // CHUNK
            off = j0 - c * CHUNK
            assert off + w <= CHUNK
            return xts[(b, c)][:, off:off + w]

        def scol(b, j0, w):
            c = j0 // CHUNK
            off = j0 - c * CHUNK
            assert off + w <= CHUNK
            return sqs[(b, c)][:, off:off + w]

        rmax2 = small.tile([P, BPC * MBLK, 2], F32)
        nc.gpsimd.memset(rmax2[:], -1.0e30)
        lsum2 = small.tile([P, BPC * MBLK, 2], F32)
        nc.gpsimd.memset(lsum2[:], 0.0)
        sqn_sb = small.tile([P, BPC * MBLK], F32)
        bias_sb = small.tile([P, BPC * MBLK], F32)
        junk = junkp.tile([P, CHUNK], BF16)

        sqnrows = {}

        def batch_setup(b):
            # borrow one psum slot for this batch's setup matmuls
            ps = psum.tile([P, CHUNK], F32, tag="ph")
            # sqn_row: 4 col-tiled K=128/M=32 matmuls -> all 128 partitions
            for c in range(4):
                nc.tensor.matmul(
                    ps[32 * c:32 * c + 32, 0:MMW],
                    ones32[:],
                    scol(b, c * MMW, MMW),
                    start=True, stop=True,
                    tile_position=(0, 32 * c),
                )
            # per-point sqn columns: 16 tiny matmuls into cols [512, 544)
            for m in range(MBLK):
                nc.tensor.matmul(
                    ps[:, MMW + 2 * m:MMW + 2 * m + 2],
                    scol(b, m * P, P),
                    ones32[:, 0:2],
                    start=True, stop=True,
                )
            srow = rowp.tile([P, MMW], BF16, tag=f"sqnrow_{b}")
            nc.vector.tensor_copy(srow[:], ps[:, 0:MMW])
            sqnrows[b] = srow
            nc.vector.tensor_copy(
                sqn_sb[:, b * MBLK:(b + 1) * MBLK],
                ps[:, MMW:MMW + 2 * MBLK].rearrange(
                    "p (c t) -> p c t", t=2)[:, :, 0:1],
            )
            # bias_i = -t/2*(sqn_i - C)
            nc.vector.tensor_scalar(
                bias_sb[:, b * MBLK:(b + 1) * MBLK],
                sqn_sb[:, b * MBLK:(b + 1) * MBLK],
                -0.5 * SOFT_T, 0.5 * SOFT_T * SOFT_C,
                op0=mybir.AluOpType.mult, op1=mybir.AluOpType.add,
            )

        for b in range(BPC):
            batch_setup(b)
            for m in range(MBLK):
                col = b * MBLK + m
                lhs_x = xcol(b, m * P, P)
                phs = []
                for h in range(2):
                    ph = psum.tile([P, N // 2], F32, tag="ph")
                    phs.append(ph)
                    for k in range(2):
                        j0 = h * (N // 2) + k * MMW
                        nc.tensor.matmul(
                            ph[:, k * MMW:(k + 1) * MMW],
                            lhs_x,
                            xcol(b, j0, MMW),
                            start=True, stop=False,
                        )
                hd = (m * P) // (N // 2)
                off = m * P - hd * (N // 2)
                nc.tensor.matmul(
                    phs[hd][:, off:off + P],
                    identneg[:],
                    ident[:],
                    start=False, stop=False,
                )
                if OFFS_TILED:
                    for h in range(2):
                        for k in range(2):
                            c = 2 * h + k
                            nc.tensor.matmul(
                                phs[h][:, k * MMW:(k + 1) * MMW],
                                neghalf[32 * c:32 * c + 1, :],
                                sqnrows[b][32 * c:32 * c + 1, :],
                                start=False, stop=True,
                                tile_position=(32 * c, 0),
                            )
                else:
                    for h in range(2):
                        for k in range(2):
                            j0 = h * (N // 2) + k * MMW
                            nc.tensor.matmul(
                                phs[h][:, k * MMW:(k + 1) * MMW],
                                neghalf[:],
                                scol(b, j0, MMW),
                                start=False, stop=True,
                            )
                for h in range(2):
                    if PAT2[col][h] == 'R':
                        nc.vector.tensor_reduce(
                            rmax2[:, col, h:h + 1], phs[h][:],
                            axis=mybir.AxisListType.X, op=mybir.AluOpType.max,
                        )
                    else:
                        nc.scalar.activation(
                            junk[:], phs[h][:],
                            mybir.ActivationFunctionType.Exp,
                            bias=bias_sb[:, col:col + 1],
                            scale=SOFT_T,
                            accum_out=lsum2[:, col, h:h + 1],
                        )

        nc.sync.dma_start(rmax2_d.ap()[:, :, :], rmax2[:])
        nc.sync.dma_start(lsum2_d.ap()[:, :, :], lsum2[:])
        nc.sync.dma_start(sqn_d.ap()[:, :], sqn_sb[:])

    nc.compile()
    return nc


def _consts():
    neghalf = np.full((P, P), -0.5, dtype=ml_dtypes.bfloat16)
    ones32 = np.ones((P, 32), dtype=ml_dtypes.bfloat16)
    identneg = (NEGBIG * np.eye(P)).astype(ml_dtypes.bfloat16)
    ident = np.eye(P, dtype=np.float32).astype(ml_dtypes.bfloat16)
    return neghalf, ones32, identneg, ident


def make_in_maps(pred_poses):
    neghalf, ones32, identneg, ident = _consts()
    in_maps = []
    for c in range(NCORES):
        xb = pred_poses[c * BPC:(c + 1) * BPC]
        xt = np.ascontiguousarray(
            xb.transpose(0, 2, 1)).astype(ml_dtypes.bfloat16)
        in_maps.append({
            "xt": xt, "neghalf": neghalf, "ones32": ones32,
            "identneg": identneg, "ident": ident,
        })
    return in_maps


def postprocess(rmax2, lsum2, sqn):
    """[128,32,2],[128,32,2],[128,32] (one core) -> nnd2 [128,32] (f64)."""
    rmax2 = np.asarray(rmax2, dtype=np.float64)
    lsum2 = np.asarray(lsum2, dtype=np.float64)
    sqn = np.asarray(sqn, dtype=np.float64)
    nnd2 = np.empty((P, BPC * MBLK), dtype=np.float64)
    for col in range(BPC * MBLK):
        halves = []
        for h in range(2):
            if PAT2[col][h] == 'R':
                halves.append(sqn[:, col] - 2.0 * rmax2[:, col, h])
            else:
                halves.append(SOFT_C - (2.0 / SOFT_T) * np.log(
                    np.maximum(lsum2[:, col, h], 1e-300)))
        nnd2[:, col] = np.minimum(halves[0], halves[1])
    return np.maximum(nnd2, 0.0)


def kernel(pred_poses: np.ndarray) -> np.ndarray:
    pred_poses = np.ascontiguousarray(np.asarray(pred_poses, dtype=np.float32))
    assert pred_poses.shape == (B, N, D)

    if "nc" not in _CACHE:
        _CACHE["nc"] = build_kernel()
    nc = _CACHE["nc"]

    in_maps = make_in_maps(pred_poses)
    res = run_bass_kernel_spmd(nc, in_maps, list(range(NCORES)))

    nnd = np.zeros((B, N), dtype=np.float64)
    for c in range(NCORES):
        r = res.results[c]
        nnd2 = postprocess(r["rmax2"], r["lsum2"], r["sqn"])
        t = np.sqrt(nnd2)                               # [128, 32]
        for bl in range(BPC):
            sub = t[:, bl * MBLK:(bl + 1) * MBLK]       # [128, 16] (p, m)
            nnd[c * BPC + bl] = sub.T.reshape(N)        # index m*128+p

    mean = nnd.mean()
    std = nnd.std(ddof=1)
    eps = 1e-8
    cv = std / max(mean, eps) if mean > eps else 0.0
    return np.stack([mean, std, cv]).astype(np.float32)


# revision 18
# speedup vs baseline: 1.4164x; 1.4164x over previous
"""Trainium2 Bass kernel for nn_DiversityMetric (batched NND diversity metric).

Math (per batch b, X = pred_poses[b] in R^{N x D}, N=2048, D=128):
    sq_dist[i,j] = ||xi||^2 + ||xj||^2 - 2 <xi,xj>, diag = inf
    nnd[i]       = sqrt(min_{j != i} sq_dist[i,j])
    out          = [mean(nnd), std(nnd, ddof=1), cv]   over all B*N points

v3 design (8 cores, 2 batches/core, bf16 matmul path):
  PE per row-strip m (128 rows x 2048 cols of the gram):
    - 4x [K=128, N=512] bf16 matmuls  v_ij = <xi, xj>         (start)
    - 1x identneg matmul: diag -= 1e6 (self-exclusion)
    - 4x row-tiled K=1 matmuls (tile_position=(32c,0), concurrent):
        v_ij += -0.5*sqn_j   -- lhsT = [-0.5] row, rhs = sqn_row chunk c
  Batch setup (borrows one PSUM slot):
    - sqn_row via 4 col-tiled K=128/M=32 matmuls (tile_position=(0,32c))
    - per-point sqn columns via 16 tiny K=128/N=2 matmuls
  Drain, strictly alternating across both PSUM-capable engines:
    'R': DVE tensor_reduce(max) per [128,1024] half -> rmax2[:,col,h]
    'S': ACT softmin: accum = sum_j exp(t*v_ij + bias_i) in one pass,
         bias_i = -t/2*(sqn_i - C).  max_j v_ij ~= (ln acc - bias)/t
         (high-dim NN gaps make exp(t*(v-vmax)) tiny), so
         nnd_i^2 = sqn_i - 2*max = C - (2/t)*ln(acc): host-side.
  PSUM: single pool of 4x [128,1024] slots (8 banks) so PE always has
  two free halves per strip -- avoids PE idle gaps (HAM p-state drops).
  Host computes nnd2/sqrt/mean/std/cv from rmax2, lsum2, sqn (f64).
"""

import numpy as np
from contextlib import ExitStack

import ml_dtypes

import concourse.bass as bass
import concourse.bacc as bacc
import concourse.mybir as mybir
import concourse.tile as tile
from concourse.bass_utils import run_bass_kernel_spmd

F32 = mybir.dt.float32
BF16 = mybir.dt.bfloat16

B, N, D = 16, 2048, 128
NCORES = 8
BPC = B // NCORES          # batches per core
P = 128                    # partitions
MBLK = N // P              # 16 row strips per batch
MMW = 512                  # matmul moving width (1 PSUM bank)
CHUNK = 1024               # xt/sq SBUF chunk width
NEGBIG = -1.0e6

SOFT_T = 2.5               # softmin sharpness
SOFT_C = 160.0             # centering constant (~E[nnd^2]); exactly cancels

OFFS_TILED = True          # row-tiled K=1 offset matmuls (else neghalf K=128)

# Per-half drain assignment: h0 -> DVE exact reduce ('R'), h1 -> ACT softmin
# ('S'), except EXTRA_R strips which send both halves to DVE (load balance).
# Per-half drain: h0 -> DVE reduce ('R'), h1 -> ACT softmin ('S'); EXTRA_R
# strips send both halves to DVE (balances 35R/29S across 64 halves).
EXTRA_R = {5, 16, 27}
PAT2 = [
    ('R', 'R') if i in EXTRA_R else ('R', 'S')
    for i in range(BPC * MBLK)
]
# Strips whose -0.5*sqn_j offset uses the full-array K=128 matmul instead of
# the row-tiled K=1 form: raises PE's pole slightly above the drain engines
# so PE stays backlogged (full clocks) and drains pace behind it.
UNTILED = {1, 5, 9, 13, 17, 21, 25, 29}
WARMUP_MM = 6

_CACHE = {}


def build_kernel():
    nc = bacc.Bacc("TRN2", target_bir_lowering=False, debug=False)

    xt_d = nc.dram_tensor("xt", [BPC, P, N], BF16, kind="ExternalInput")
    neghalf_d = nc.dram_tensor("neghalf", [P, P], BF16, kind="ExternalInput")
    ones_d = nc.dram_tensor("ones32", [P, 32], BF16, kind="ExternalInput")
    identneg_d = nc.dram_tensor("identneg", [P, P], BF16, kind="ExternalInput")
    ident_d = nc.dram_tensor("ident", [P, P], BF16, kind="ExternalInput")
    rmax2_d = nc.dram_tensor("rmax2", [P, BPC * MBLK, 2], F32,
                             kind="ExternalOutput")
    lsum2_d = nc.dram_tensor("lsum2", [P, BPC * MBLK, 2], F32,
                             kind="ExternalOutput")
    sqn_d = nc.dram_tensor("sqn", [P, BPC * MBLK], F32, kind="ExternalOutput")

    with tile.TileContext(nc) as tc, ExitStack() as ctx:
        const = ctx.enter_context(tc.tile_pool(name="const", bufs=1))
        xpool = ctx.enter_context(tc.tile_pool(name="x", bufs=1))
        spool = ctx.enter_context(tc.tile_pool(name="s", bufs=1))
        small = ctx.enter_context(tc.tile_pool(name="small", bufs=1))
        rowp = ctx.enter_context(tc.tile_pool(name="rowp", bufs=1))
        junkp = ctx.enter_context(tc.tile_pool(name="junk", bufs=1))
        psum = ctx.enter_context(tc.tile_pool(name="psum", bufs=4, space="PSUM"))

        NCHUNK = N // CHUNK
        xts = {}
        sqs = {}

        def load_chunk(b, c):
            xtile = xpool.tile([P, CHUNK], BF16, tag=f"xt_{b}_{c}")
            nc.sync.dma_start(
                xtile[:], xt_d.ap()[b, :, c * CHUNK:(c + 1) * CHUNK]
            )
            xts[(b, c)] = xtile
            stile = spool.tile([P, CHUNK], BF16, tag=f"sq_{b}_{c}")
            if b == 0 and c == 0:
                nc.scalar.square(stile[:], xtile[:])
            elif b == 0:
                nc.vector.tensor_tensor(
                    stile[:], xtile[:], xtile[:], op=mybir.AluOpType.mult
                )
            else:
                nc.gpsimd.tensor_mul(stile[:], xtile[:], xtile[:])
            sqs[(b, c)] = stile

        load_chunk(0, 0)

        neghalf = const.tile([P, P], BF16)
        nc.scalar.dma_start(neghalf[:], neghalf_d.ap())
        ones32 = const.tile([P, 32], BF16)
        nc.scalar.dma_start(ones32[:], ones_d.ap())
        identneg = const.tile([P, P], BF16)
        nc.scalar.dma_start(identneg[:], identneg_d.ap())
        ident = const.tile([P, P], BF16)
        nc.scalar.dma_start(ident[:], ident_d.ap())

        for b in range(BPC):
            for c in range(NCHUNK):
                if (b, c) != (0, 0):
                    load_chunk(b, c)

        def xcol(b, j0, w):
            c = j0 // CHUNK
            off = j0 - c * CHUNK
            assert off + w <= CHUNK
            return xts[(b, c)][:, off:off + w]

        def scol(b, j0, w):
            c = j0 // CHUNK
            off = j0 - c * CHUNK
            assert off + w <= CHUNK
            return sqs[(b, c)][:, off:off + w]

        rmax2 = small.tile([P, BPC * MBLK, 2], F32)
        nc.gpsimd.memset(rmax2[:], -1.0e30)
        lsum2 = small.tile([P, BPC * MBLK, 2], F32)
        nc.gpsimd.memset(lsum2[:], 0.0)
        sqn_sb = small.tile([P, BPC * MBLK], F32)
        bias_sb = small.tile([P, BPC * MBLK], F32)
        junk = junkp.tile([P, CHUNK], BF16)

        # PE warmup: a few throwaway matmuls as soon as the first chunk
        # lands, so the PE clock governor ramps to full speed before the
        # real gram stream starts.
        if WARMUP_MM:
            pw = psum.tile([P, N // 2], F32, tag="ph")
            for w in range(WARMUP_MM):
                nc.tensor.matmul(
                    pw[:, 0:MMW], ident[:], xts[(0, 0)][:, 0:MMW],
                    start=True, stop=True,
                )
            nc.vector.tensor_copy(junk[:, 0:2], pw[:, 0:2])

        sqnrows = {}

        def batch_setup(b):
            # borrow one psum slot for this batch's setup matmuls
            ps = psum.tile([P, CHUNK], F32, tag="ph")
            # sqn_row: 4 col-tiled K=128/M=32 matmuls -> all 128 partitions
            for c in range(4):
                nc.tensor.matmul(
                    ps[32 * c:32 * c + 32, 0:MMW],
                    ones32[:],
                    scol(b, c * MMW, MMW),
                    start=True, stop=True,
                    tile_position=(0, 32 * c),
                )
            # per-point sqn columns: 16 tiny matmuls into cols [512, 544)
            for m in range(MBLK):
                nc.tensor.matmul(
                    ps[:, MMW + 2 * m:MMW + 2 * m + 2],
                    scol(b, m * P, P),
                    ones32[:, 0:2],
                    start=True, stop=True,
                )
            srow = rowp.tile([P, MMW], BF16, tag=f"sqnrow_{b}")
            nc.vector.tensor_copy(srow[:], ps[:, 0:MMW])
            sqnrows[b] = srow
            nc.vector.tensor_copy(
                sqn_sb[:, b * MBLK:(b + 1) * MBLK],
                ps[:, MMW:MMW + 2 * MBLK].rearrange(
                    "p (c t) -> p c t", t=2)[:, :, 0:1],
            )
            # bias_i = -t/2*(sqn_i - C)
            nc.vector.tensor_scalar(
                bias_sb[:, b * MBLK:(b + 1) * MBLK],
                sqn_sb[:, b * MBLK:(b + 1) * MBLK],
                -0.5 * SOFT_T, 0.5 * SOFT_T * SOFT_C,
                op0=mybir.AluOpType.mult, op1=mybir.AluOpType.add,
            )

        for b in range(BPC):
            batch_setup(b)
            for m in range(MBLK):
                col = b * MBLK + m
                lhs_x = xcol(b, m * P, P)
                phs = []
                for h in range(2):
                    ph = psum.tile([P, N // 2], F32, tag="ph")
                    phs.append(ph)
                    for k in range(2):
                        j0 = h * (N // 2) + k * MMW
                        nc.tensor.matmul(
                            ph[:, k * MMW:(k + 1) * MMW],
                            lhs_x,
                            xcol(b, j0, MMW),
                            start=True, stop=False,
                        )
                hd = (m * P) // (N // 2)
                off = m * P - hd * (N // 2)
                nc.tensor.matmul(
                    phs[hd][:, off:off + P],
                    identneg[:],
                    ident[:],
                    start=False, stop=False,
                )
                if OFFS_TILED and col not in UNTILED:
                    for h in range(2):
                        for k in range(2):
                            c = 2 * h + k
                            nc.tensor.matmul(
                                phs[h][:, k * MMW:(k + 1) * MMW],
                                neghalf[32 * c:32 * c + 1, :],
                                sqnrows[b][32 * c:32 * c + 1, :],
                                start=False, stop=True,
                                tile_position=(32 * c, 0),
                            )
                else:
                    for h in range(2):
                        for k in range(2):
                            j0 = h * (N // 2) + k * MMW
                            nc.tensor.matmul(
                                phs[h][:, k * MMW:(k + 1) * MMW],
                                neghalf[:],
                                scol(b, j0, MMW),
                                start=False, stop=True,
                            )
                for h in range(2):
                    if PAT2[col][h] == 'R':
                        nc.vector.tensor_reduce(
                            rmax2[:, col, h:h + 1], phs[h][:],
                            axis=mybir.AxisListType.X, op=mybir.AluOpType.max,
                        )
                    else:
                        nc.scalar.activation(
                            junk[:], phs[h][:],
                            mybir.ActivationFunctionType.Exp,
                            bias=bias_sb[:, col:col + 1],
                            scale=SOFT_T,
                            accum_out=lsum2[:, col, h:h + 1],
                        )

        nc.sync.dma_start(rmax2_d.ap()[:, :, :], rmax2[:])
        nc.sync.dma_start(lsum2_d.ap()[:, :, :], lsum2[:])
        nc.sync.dma_start(sqn_d.ap()[:, :], sqn_sb[:])

    nc.compile()
    return nc


def _consts():
    neghalf = np.full((P, P), -0.5, dtype=ml_dtypes.bfloat16)
    ones32 = np.ones((P, 32), dtype=ml_dtypes.bfloat16)
    identneg = (NEGBIG * np.eye(P)).astype(ml_dtypes.bfloat16)
    ident = np.eye(P, dtype=np.float32).astype(ml_dtypes.bfloat16)
    return neghalf, ones32, identneg, ident


def make_in_maps(pred_poses):
    neghalf, ones32, identneg, ident = _consts()
    in_maps = []
    for c in range(NCORES):
        xb = pred_poses[c * BPC:(c + 1) * BPC]
        xt = np.ascontiguousarray(
            xb.transpose(0, 2, 1)).astype(ml_dtypes.bfloat16)
        in_maps.append({
            "xt": xt, "neghalf": neghalf, "ones32": ones32,
            "identneg": identneg, "ident": ident,
        })
    return in_maps


def postprocess(rmax2, lsum2, sqn):
    """[128,32,2],[128,32,2],[128,32] (one core) -> nnd2 [128,32] (f64)."""
    rmax2 = np.asarray(rmax2, dtype=np.float64)
    lsum2 = np.asarray(lsum2, dtype=np.float64)
    sqn = np.asarray(sqn, dtype=np.float64)
    nnd2 = np.empty((P, BPC * MBLK), dtype=np.float64)
    for col in range(BPC * MBLK):
        halves = []
        for h in range(2):
            if PAT2[col][h] == 'R':
                halves.append(sqn[:, col] - 2.0 * rmax2[:, col, h])
            else:
                halves.append(SOFT_C - (2.0 / SOFT_T) * np.log(
                    np.maximum(lsum2[:, col, h], 1e-300)))
        nnd2[:, col] = np.minimum(halves[0], halves[1])
    return np.maximum(nnd2, 0.0)


def kernel(pred_poses: np.ndarray) -> np.ndarray:
    pred_poses = np.ascontiguousarray(np.asarray(pred_poses, dtype=np.float32))
    assert pred_poses.shape == (B, N, D)

    if "nc" not in _CACHE:
        _CACHE["nc"] = build_kernel()
    nc = _CACHE["nc"]

    in_maps = make_in_maps(pred_poses)
    res = run_bass_kernel_spmd(nc, in_maps, list(range(NCORES)))

    nnd = np.zeros((B, N), dtype=np.float64)
    for c in range(NCORES):
        r = res.results[c]
        nnd2 = postprocess(r["rmax2"], r["lsum2"], r["sqn"])
        t = np.sqrt(nnd2)                               # [128, 32]
        for bl in range(BPC):
            sub = t[:, bl * MBLK:(bl + 1) * MBLK]       # [128, 16] (p, m)
            nnd[c * BPC + bl] = sub.T.reshape(N)        # index m*128+p

    mean = nnd.mean()
    std = nnd.std(ddof=1)
    eps = 1e-8
    cv = std / max(mean, eps) if mean > eps else 0.0
    return np.stack([mean, std, cv]).astype(np.float32)


# revision 20
# speedup vs baseline: 1.5120x; 1.0674x over previous
"""Trainium2 Bass kernel for nn_DiversityMetric (batched NND diversity metric).

Math (per batch b, X = pred_poses[b] in R^{N x D}, N=2048, D=128):
    sq_dist[i,j] = ||xi||^2 + ||xj||^2 - 2 <xi,xj>, diag = inf
    nnd[i]       = sqrt(min_{j != i} sq_dist[i,j])
    out          = [mean(nnd), std(nnd, ddof=1), cv]   over all B*N points

v7 design (8 cores, 2 batches/core):
  Inputs ship as fp8e4 (halves the load DMA; K=128 fp8 matmuls run at
  bf16 speed and the ~2% element quantization perturbs d^2 by ~0.2%,
  far under the 2e-2 gate).  Squares are computed on-device in bf16.
  PE per row-strip m (128 rows x 2048 cols of the gram):
    - 4x [K=128, N=512] fp8 matmuls  v_ij = <xi, xj>          (start)
    - 1x identneg matmul: diag -= 1e6 (self-exclusion)
    - offsets v_ij += -0.5*sqn_j: row-tiled K=1 matmuls
      (tile_position=(32c,0), ~4x concurrent) for most strips; a few
      UNTILED strips use the full-array K=128 form to keep PE the
      clearly-longest engine (PE-backlog => full DVFS clocks).
  Batch setup (borrows one PSUM slot): sqn_row via 4 col-tiled
  K=128/M=32 matmuls; per-point sqn via 16 tiny matmuls.
  Drain, per PSUM half [128,1024]: 'R' halves -> DVE tensor_reduce(max)
  -> rmax2; 'S' halves -> ACT softmin exp+accum with per-row bias
  bias_i = -t/2*(sqn_i - C), so nnd_i^2 = C - (2/t)*ln(accum).
  PE warmup matmuls at the start ramp the clock governor before the
  real stream.  Host does ln/sqrt/mean/std/cv in f64.
"""

import numpy as np
from contextlib import ExitStack

import ml_dtypes

import concourse.bass as bass
import concourse.bacc as bacc
import concourse.mybir as mybir
import concourse.tile as tile
from concourse.bass_utils import run_bass_kernel_spmd

F32 = mybir.dt.float32
BF16 = mybir.dt.bfloat16
FP8 = mybir.dt.float8e4

B, N, D = 16, 2048, 128
NCORES = 8
BPC = B // NCORES          # batches per core
P = 128                    # partitions
MBLK = N // P              # 16 row strips per batch
MMW = 512                  # matmul moving width (1 PSUM bank)
CHUNK = 1024               # xt/sq SBUF chunk width
NEGBIG = -1.0e6

SOFT_T = 2.5               # softmin sharpness
SOFT_C = 160.0             # centering constant (~E[nnd^2]); exactly cancels

# Per-half drain: h0 -> DVE reduce ('R'), h1 -> ACT softmin ('S'); EXTRA_R
# strips send both halves to DVE (balances 34R/30S across 64 halves).
EXTRA_R = {8, 24}
PAT2 = [
    ('R', 'R') if i in EXTRA_R else ('R', 'S')
    for i in range(BPC * MBLK)
]
# Strips whose offset uses the full-array K=128 matmul: keeps PE the pole.
UNTILED = {3, 11, 19, 27}
WARMUP_MM = 4

_CACHE = {}


def build_kernel():
    nc = bacc.Bacc("TRN2", target_bir_lowering=False, debug=False)

    xt_d = nc.dram_tensor("xt", [BPC, P, N], FP8, kind="ExternalInput")
    neghalf_d = nc.dram_tensor("neghalf", [P, P], BF16, kind="ExternalInput")
    ones_d = nc.dram_tensor("ones32", [P, 32], BF16, kind="ExternalInput")
    identneg_d = nc.dram_tensor("identneg", [P, P], BF16, kind="ExternalInput")
    ident_d = nc.dram_tensor("ident", [P, P], BF16, kind="ExternalInput")
    rmax2_d = nc.dram_tensor("rmax2", [P, BPC * MBLK, 2], F32,
                             kind="ExternalOutput")
    lsum2_d = nc.dram_tensor("lsum2", [P, BPC * MBLK, 2], F32,
                             kind="ExternalOutput")
    sqn_d = nc.dram_tensor("sqn", [P, BPC * MBLK], F32, kind="ExternalOutput")

    with tile.TileContext(nc) as tc, ExitStack() as ctx:
        const = ctx.enter_context(tc.tile_pool(name="const", bufs=1))
        xpool = ctx.enter_context(tc.tile_pool(name="x", bufs=1))
        spool = ctx.enter_context(tc.tile_pool(name="s", bufs=1))
        small = ctx.enter_context(tc.tile_pool(name="small", bufs=1))
        rowp = ctx.enter_context(tc.tile_pool(name="rowp", bufs=1))
        junkp = ctx.enter_context(tc.tile_pool(name="junk", bufs=1))
        psum = ctx.enter_context(tc.tile_pool(name="psum", bufs=4, space="PSUM"))

        NCHUNK = N // CHUNK
        xts = {}
        sqs = {}

        def load_chunk(b, c):
            xtile = xpool.tile([P, CHUNK], FP8, tag=f"xt_{b}_{c}")
            nc.sync.dma_start(
                xtile[:], xt_d.ap()[b, :, c * CHUNK:(c + 1) * CHUNK]
            )
            xts[(b, c)] = xtile
            stile = spool.tile([P, CHUNK], BF16, tag=f"sq_{b}_{c}")
            if b == 0 and c == 0:
                nc.scalar.square(stile[:], xtile[:])
            else:
                nc.vector.tensor_tensor(
                    stile[:], xtile[:], xtile[:], op=mybir.AluOpType.mult
                )
            sqs[(b, c)] = stile

        load_chunk(0, 0)

        neghalf = const.tile([P, P], BF16)
        nc.scalar.dma_start(neghalf[:], neghalf_d.ap())
        ones32 = const.tile([P, 32], BF16)
        nc.scalar.dma_start(ones32[:], ones_d.ap())
        identneg = const.tile([P, P], BF16)
        nc.scalar.dma_start(identneg[:], identneg_d.ap())
        ident = const.tile([P, P], BF16)
        nc.scalar.dma_start(ident[:], ident_d.ap())

        for b in range(BPC):
            for c in range(NCHUNK):
                if (b, c) != (0, 0):
                    load_chunk(b, c)

        def xcol(b, j0, w):
            c = j0 // CHUNK
            off = j0 - c * CHUNK
            assert off + w <= CHUNK
            return xts[(b, c)][:, off:off + w]

        def scol(b, j0, w):
            c = j0 // CHUNK
            off = j0 - c * CHUNK
            assert off + w <= CHUNK
            return sqs[(b, c)][:, off:off + w]

        rmax2 = small.tile([P, BPC * MBLK, 2], F32)
        nc.gpsimd.memset(rmax2[:], -1.0e30)
        lsum2 = small.tile([P, BPC * MBLK, 2], F32)
        nc.gpsimd.memset(lsum2[:], 0.0)
        sqn_sb = small.tile([P, BPC * MBLK], F32)
        bias_sb = small.tile([P, BPC * MBLK], F32)
        junk = junkp.tile([P, CHUNK], BF16)

        # PE warmup: throwaway matmuls as soon as the first chunk lands,
        # so the clock governor ramps before the real gram stream.
        if WARMUP_MM:
            pw = psum.tile([P, N // 2], F32, tag="ph")
            for w in range(WARMUP_MM):
                nc.tensor.matmul(
                    pw[:, 0:MMW], xts[(0, 0)][:, 0:P], xts[(0, 0)][:, 0:MMW],
                    start=True, stop=True,
                )
            nc.vector.tensor_copy(junk[:, 0:2], pw[:, 0:2])

        sqnrows = {}

        def batch_setup(b):
            # borrow one psum slot for this batch's setup matmuls
            ps = psum.tile([P, CHUNK], F32, tag="ph")
            # sqn_row: 4 col-tiled K=128/M=32 matmuls -> all 128 partitions
            for c in range(4):
                nc.tensor.matmul(
                    ps[32 * c:32 * c + 32, 0:MMW],
                    ones32[:],
                    scol(b, c * MMW, MMW),
                    start=True, stop=True,
                    tile_position=(0, 32 * c),
                )
            # per-point sqn columns: 16 tiny matmuls into cols [512, 544)
            for m in range(MBLK):
                nc.tensor.matmul(
                    ps[:, MMW + 2 * m:MMW + 2 * m + 2],
                    scol(b, m * P, P),
                    ones32[:, 0:2],
                    start=True, stop=True,
                )
            srow = rowp.tile([P, MMW], BF16, tag=f"sqnrow_{b}")
            nc.scalar.copy(srow[:], ps[:, 0:MMW])
            sqnrows[b] = srow
            nc.vector.tensor_copy(
                sqn_sb[:, b * MBLK:(b + 1) * MBLK],
                ps[:, MMW:MMW + 2 * MBLK].rearrange(
                    "p (c t) -> p c t", t=2)[:, :, 0:1],
            )
            # bias_i = -t/2*(sqn_i - C)
            nc.vector.tensor_scalar(
                bias_sb[:, b * MBLK:(b + 1) * MBLK],
                sqn_sb[:, b * MBLK:(b + 1) * MBLK],
                -0.5 * SOFT_T, 0.5 * SOFT_T * SOFT_C,
                op0=mybir.AluOpType.mult, op1=mybir.AluOpType.add,
            )

        def emit_strip_pe(b, m):
            lhs_x = xcol(b, m * P, P)
            phs = []
            for h in range(2):
                ph = psum.tile([P, N // 2], F32, tag="ph")
                phs.append(ph)
                for k in range(2):
                    j0 = h * (N // 2) + k * MMW
                    nc.tensor.matmul(
                        ph[:, k * MMW:(k + 1) * MMW],
                        lhs_x,
                        xcol(b, j0, MMW),
                        start=True, stop=False,
                    )
            hd = (m * P) // (N // 2)
            off = m * P - hd * (N // 2)
            nc.tensor.matmul(
                phs[hd][:, off:off + P],
                identneg[:],
                ident[:],
                start=False, stop=False,
            )
            return phs

        def emit_strip_offs(b, m, phs):
            col = b * MBLK + m
            if col not in UNTILED:
                for h in range(2):
                    for k in range(2):
                        c = 2 * h + k
                        nc.tensor.matmul(
                            phs[h][:, k * MMW:(k + 1) * MMW],
                            neghalf[32 * c:32 * c + 1, :],
                            sqnrows[b][32 * c:32 * c + 1, :],
                            start=False, stop=True,
                            tile_position=(32 * c, 0),
                        )
            else:
                for h in range(2):
                    for k in range(2):
                        j0 = h * (N // 2) + k * MMW
                        nc.tensor.matmul(
                            phs[h][:, k * MMW:(k + 1) * MMW],
                            neghalf[:],
                            scol(b, j0, MMW),
                            start=False, stop=True,
                        )

        def emit_strip_drain(b, m, phs):
            col = b * MBLK + m
            for h in range(2):
                if PAT2[col][h] == 'R':
                    nc.vector.tensor_reduce(
                        rmax2[:, col, h:h + 1], phs[h][:],
                        axis=mybir.AxisListType.X, op=mybir.AluOpType.max,
                    )
                else:
                    nc.scalar.activation(
                        junk[:], phs[h][:],
                        mybir.ActivationFunctionType.Exp,
                        bias=bias_sb[:, col:col + 1],
                        scale=SOFT_T,
                        accum_out=lsum2[:, col, h:h + 1],
                    )

        # batch 0: strips 0-1's gram matmuls are emitted before the batch
        # setup so PE has real work while sq/sqn_row are still cooking.
        NPRE = 1
        pre = [emit_strip_pe(0, m) for m in range(NPRE)]
        batch_setup(0)
        for m in range(NPRE):
            emit_strip_offs(0, m, pre[m])
            emit_strip_drain(0, m, pre[m])
        for m in range(NPRE, MBLK):
            phs = emit_strip_pe(0, m)
            emit_strip_offs(0, m, phs)
            emit_strip_drain(0, m, phs)
        batch_setup(1)
        for m in range(MBLK):
            phs = emit_strip_pe(1, m)
            emit_strip_offs(1, m, phs)
            emit_strip_drain(1, m, phs)

        nc.sync.dma_start(rmax2_d.ap()[:, :, :], rmax2[:])
        nc.sync.dma_start(lsum2_d.ap()[:, :, :], lsum2[:])
        nc.sync.dma_start(sqn_d.ap()[:, :], sqn_sb[:])

    nc.compile()
    return nc


def _consts():
    neghalf = np.full((P, P), -0.5, dtype=ml_dtypes.bfloat16)
    ones32 = np.ones((P, 32), dtype=ml_dtypes.bfloat16)
    identneg = (NEGBIG * np.eye(P)).astype(ml_dtypes.bfloat16)
    ident = np.eye(P, dtype=np.float32).astype(ml_dtypes.bfloat16)
    return neghalf, ones32, identneg, ident


def make_in_maps(pred_poses):
    neghalf, ones32, identneg, ident = _consts()
    in_maps = []
    for c in range(NCORES):
        xb = pred_poses[c * BPC:(c + 1) * BPC]
        xt = np.ascontiguousarray(
            xb.transpose(0, 2, 1)).astype(ml_dtypes.float8_e4m3)
        in_maps.append({
            "xt": xt, "neghalf": neghalf, "ones32": ones32,
            "identneg": identneg, "ident": ident,
        })
    return in_maps


def postprocess(rmax2, lsum2, sqn):
    """[128,32,2],[128,32,2],[128,32] (one core) -> nnd2 [128,32] (f64)."""
    rmax2 = np.asarray(rmax2, dtype=np.float64)
    lsum2 = np.asarray(lsum2, dtype=np.float64)
    sqn = np.asarray(sqn, dtype=np.float64)
    nnd2 = np.empty((P, BPC * MBLK), dtype=np.float64)
    for col in range(BPC * MBLK):
        halves = []
        for h in range(2):
            if PAT2[col][h] == 'R':
                halves.append(sqn[:, col] - 2.0 * rmax2[:, col, h])
            else:
                halves.append(SOFT_C - (2.0 / SOFT_T) * np.log(
                    np.maximum(lsum2[:, col, h], 1e-300)))
        nnd2[:, col] = np.minimum(halves[0], halves[1])
    return np.maximum(nnd2, 0.0)


def kernel(pred_poses: np.ndarray) -> np.ndarray:
    pred_poses = np.ascontiguousarray(np.asarray(pred_poses, dtype=np.float32))
    assert pred_poses.shape == (B, N, D)

    if "nc" not in _CACHE:
        _CACHE["nc"] = build_kernel()
    nc = _CACHE["nc"]

    in_maps = make_in_maps(pred_poses)
    res = run_bass_kernel_spmd(nc, in_maps, list(range(NCORES)))

    nnd = np.zeros((B, N), dtype=np.float64)
    for c in range(NCORES):
        r = res.results[c]
        nnd2 = postprocess(r["rmax2"], r["lsum2"], r["sqn"])
        t = np.sqrt(nnd2)                               # [128, 32]
        for bl in range(BPC):
            sub = t[:, bl * MBLK:(bl + 1) * MBLK]       # [128, 16] (p, m)
            nnd[c * BPC + bl] = sub.T.reshape(N)        # index m*128+p

    mean = nnd.mean()
    std = nnd.std(ddof=1)
    eps = 1e-8
    cv = std / max(mean, eps) if mean > eps else 0.0
    return np.stack([mean, std, cv]).astype(np.float32)


# revision 23
# speedup vs baseline: 1.5172x; 1.0034x over previous
"""Trainium2 Bass kernel for nn_DiversityMetric (batched NND diversity metric).

Math (per batch b, X = pred_poses[b] in R^{N x D}, N=2048, D=128):
    sq_dist[i,j] = ||xi||^2 + ||xj||^2 - 2 <xi,xj>, diag = inf
    nnd[i]       = sqrt(min_{j != i} sq_dist[i,j])
    out          = [mean(nnd), std(nnd, ddof=1), cv]   over all B*N points

v7 design (8 cores, 2 batches/core):
  Inputs ship as fp8e4 (halves the load DMA; K=128 fp8 matmuls run at
  bf16 speed and the ~2% element quantization perturbs d^2 by ~0.2%,
  far under the 2e-2 gate).  Squares are computed on-device in bf16.
  PE per row-strip m (128 rows x 2048 cols of the gram):
    - 4x [K=128, N=512] fp8 matmuls  v_ij = <xi, xj>          (start)
    - 1x identneg matmul: diag -= 1e6 (self-exclusion)
    - offsets v_ij += -0.5*sqn_j: row-tiled K=1 matmuls
      (tile_position=(32c,0), ~4x concurrent) for most strips; a few
      UNTILED strips use the full-array K=128 form to keep PE the
      clearly-longest engine (PE-backlog => full DVFS clocks).
  Batch setup (borrows one PSUM slot): sqn_row via 4 col-tiled
  K=128/M=32 matmuls; per-point sqn via 16 tiny matmuls.
  Drain, per PSUM half [128,1024]: 'R' halves -> DVE tensor_reduce(max)
  -> rmax2; 'S' halves -> ACT softmin exp+accum with per-row bias
  bias_i = -t/2*(sqn_i - C), so nnd_i^2 = C - (2/t)*ln(accum).
  PE warmup matmuls at the start ramp the clock governor before the
  real stream.  Host does ln/sqrt/mean/std/cv in f64.
"""

import numpy as np
from contextlib import ExitStack

import ml_dtypes

import concourse.bass as bass
import concourse.bacc as bacc
import concourse.mybir as mybir
import concourse.tile as tile
from concourse.bass_utils import run_bass_kernel_spmd

F32 = mybir.dt.float32
BF16 = mybir.dt.bfloat16
FP8 = mybir.dt.float8e4

B, N, D = 16, 2048, 128
NCORES = 8
BPC = B // NCORES          # batches per core
P = 128                    # partitions
MBLK = N // P              # 16 row strips per batch
MMW = 512                  # matmul moving width (1 PSUM bank)
CHUNK = 1024               # xt/sq SBUF chunk width
NEGBIG = -1.0e6

SOFT_T = 2.5               # softmin sharpness
SOFT_C = 160.0             # centering constant (~E[nnd^2]); exactly cancels

# Per-half drain: h0 -> DVE reduce ('R'), h1 -> ACT softmin ('S'); EXTRA_R
# strips send both halves to DVE (balances 34R/30S across 64 halves).
EXTRA_R = {8, 24}
PAT2 = [
    ('R', 'R') if i in EXTRA_R else ('R', 'S')
    for i in range(BPC * MBLK)
]
# Strips whose offset uses the full-array K=128 matmul: keeps PE the pole.
UNTILED = {11, 27}
WARMUP_MM = 4

_CACHE = {}


def build_kernel():
    nc = bacc.Bacc("TRN2", target_bir_lowering=False, debug=False)

    xt_d = nc.dram_tensor("xt", [BPC, P, N], FP8, kind="ExternalInput")
    neghalf_d = nc.dram_tensor("neghalf", [P, P], BF16, kind="ExternalInput")
    ones_d = nc.dram_tensor("ones32", [P, 32], BF16, kind="ExternalInput")
    identneg_d = nc.dram_tensor("identneg", [P, P], BF16, kind="ExternalInput")
    ident_d = nc.dram_tensor("ident", [P, P], BF16, kind="ExternalInput")
    rmax2_d = nc.dram_tensor("rmax2", [P, BPC * MBLK, 2], F32,
                             kind="ExternalOutput")
    lsum2_d = nc.dram_tensor("lsum2", [P, BPC * MBLK, 2], F32,
                             kind="ExternalOutput")
    sqn_d = nc.dram_tensor("sqn", [P, BPC * MBLK], F32, kind="ExternalOutput")

    with tile.TileContext(nc) as tc, ExitStack() as ctx:
        const = ctx.enter_context(tc.tile_pool(name="const", bufs=1))
        xpool = ctx.enter_context(tc.tile_pool(name="x", bufs=1))
        spool = ctx.enter_context(tc.tile_pool(name="s", bufs=1))
        small = ctx.enter_context(tc.tile_pool(name="small", bufs=1))
        rowp = ctx.enter_context(tc.tile_pool(name="rowp", bufs=1))
        junkp = ctx.enter_context(tc.tile_pool(name="junk", bufs=1))
        psum = ctx.enter_context(tc.tile_pool(name="psum", bufs=4, space="PSUM"))

        xts = {}
        sqs = {}

        def load_batch(b):
            # one [128, 2048] fp8 DMA per batch: 2KB contiguous per
            # partition line (the 1KB chunked form ran the DMA rings at
            # poor line efficiency and serialized the start).
            xtile = xpool.tile([P, N], FP8, tag=f"xt_{b}")
            nc.sync.dma_start(xtile[:], xt_d.ap()[b, :, :])
            xts[b] = xtile
            stile = spool.tile([P, N], BF16, tag=f"sq_{b}")
            sqs[b] = stile
            for c in range(2):
                half = slice(c * CHUNK, (c + 1) * CHUNK)
                if b == 0 and c == 0:
                    nc.scalar.square(stile[:, half], xtile[:, half])
                elif b == 0:
                    nc.vector.tensor_tensor(
                        stile[:, half], xtile[:, half], xtile[:, half],
                        op=mybir.AluOpType.mult,
                    )
                else:
                    nc.gpsimd.tensor_mul(
                        stile[:, half], xtile[:, half], xtile[:, half]
                    )

        load_batch(0)

        neghalf = const.tile([P, P], BF16)
        nc.scalar.dma_start(neghalf[:], neghalf_d.ap())
        ones32 = const.tile([P, 32], BF16)
        nc.scalar.dma_start(ones32[:], ones_d.ap())
        identneg = const.tile([P, P], BF16)
        nc.scalar.dma_start(identneg[:], identneg_d.ap())
        ident = const.tile([P, P], BF16)
        nc.scalar.dma_start(ident[:], ident_d.ap())

        load_batch(1)

        def xcol(b, j0, w):
            return xts[b][:, j0:j0 + w]

        def scol(b, j0, w):
            return sqs[b][:, j0:j0 + w]

        rmax2 = small.tile([P, BPC * MBLK, 2], F32)
        nc.gpsimd.memset(rmax2[:], -1.0e30)
        lsum2 = small.tile([P, BPC * MBLK, 2], F32)
        nc.gpsimd.memset(lsum2[:], 0.0)
        sqn_sb = small.tile([P, BPC * MBLK], F32)
        bias_sb = small.tile([P, BPC * MBLK], F32)
        junk = junkp.tile([P, CHUNK], BF16)

        # PE warmup: throwaway matmuls as soon as the first chunk lands,
        # so the clock governor ramps before the real gram stream.
        if WARMUP_MM:
            pw = psum.tile([P, N // 2], F32, tag="ph")
            for w in range(WARMUP_MM):
                nc.tensor.matmul(
                    pw[:, 0:MMW], xts[0][:, 0:P], xts[0][:, 0:MMW],
                    start=True, stop=True,
                )
            nc.vector.tensor_copy(junk[:, 0:2], pw[:, 0:2])

        sqnrows = {}

        def batch_setup(b):
            # borrow one psum slot for this batch's setup matmuls
            ps = psum.tile([P, CHUNK], F32, tag="ph")
            # sqn_row: 4 col-tiled K=128/M=32 matmuls -> all 128 partitions
            for c in range(4):
                nc.tensor.matmul(
                    ps[32 * c:32 * c + 32, 0:MMW],
                    ones32[:],
                    scol(b, c * MMW, MMW),
                    start=True, stop=True,
                    tile_position=(0, 32 * c),
                )
            # per-point sqn columns: 16 tiny matmuls into cols [512, 544)
            for m in range(MBLK):
                nc.tensor.matmul(
                    ps[:, MMW + 2 * m:MMW + 2 * m + 2],
                    scol(b, m * P, P),
                    ones32[:, 0:2],
                    start=True, stop=True,
                )
            srow = rowp.tile([P, MMW], BF16, tag=f"sqnrow_{b}")
            nc.scalar.copy(srow[:], ps[:, 0:MMW])
            sqnrows[b] = srow
            nc.vector.tensor_copy(
                sqn_sb[:, b * MBLK:(b + 1) * MBLK],
                ps[:, MMW:MMW + 2 * MBLK].rearrange(
                    "p (c t) -> p c t", t=2)[:, :, 0:1],
            )
            # bias_i = -t/2*(sqn_i - C)
            nc.vector.tensor_scalar(
                bias_sb[:, b * MBLK:(b + 1) * MBLK],
                sqn_sb[:, b * MBLK:(b + 1) * MBLK],
                -0.5 * SOFT_T, 0.5 * SOFT_T * SOFT_C,
                op0=mybir.AluOpType.mult, op1=mybir.AluOpType.add,
            )

        def emit_strip_pe(b, m):
            lhs_x = xcol(b, m * P, P)
            phs = []
            for h in range(2):
                ph = psum.tile([P, N // 2], F32, tag="ph")
                phs.append(ph)
                for k in range(2):
                    j0 = h * (N // 2) + k * MMW
                    nc.tensor.matmul(
                        ph[:, k * MMW:(k + 1) * MMW],
                        lhs_x,
                        xcol(b, j0, MMW),
                        start=True, stop=False,
                    )
            hd = (m * P) // (N // 2)
            off = m * P - hd * (N // 2)
            nc.tensor.matmul(
                phs[hd][:, off:off + P],
                identneg[:],
                ident[:],
                start=False, stop=False,
            )
            return phs

        def emit_strip_offs(b, m, phs):
            col = b * MBLK + m
            if col not in UNTILED:
                for h in range(2):
                    for k in range(2):
                        c = 2 * h + k
                        nc.tensor.matmul(
                            phs[h][:, k * MMW:(k + 1) * MMW],
                            neghalf[32 * c:32 * c + 1, :],
                            sqnrows[b][32 * c:32 * c + 1, :],
                            start=False, stop=True,
                            tile_position=(32 * c, 0),
                        )
            else:
                for h in range(2):
                    for k in range(2):
                        j0 = h * (N // 2) + k * MMW
                        nc.tensor.matmul(
                            phs[h][:, k * MMW:(k + 1) * MMW],
                            neghalf[:],
                            scol(b, j0, MMW),
                            start=False, stop=True,
                        )

        def emit_strip_drain(b, m, phs):
            col = b * MBLK + m
            for h in range(2):
                if PAT2[col][h] == 'R':
                    nc.vector.tensor_reduce(
                        rmax2[:, col, h:h + 1], phs[h][:],
                        axis=mybir.AxisListType.X, op=mybir.AluOpType.max,
                    )
                else:
                    nc.scalar.activation(
                        junk[:], phs[h][:],
                        mybir.ActivationFunctionType.Exp,
                        bias=bias_sb[:, col:col + 1],
                        scale=SOFT_T,
                        accum_out=lsum2[:, col, h:h + 1],
                    )

        # batch 0: strips 0-1's gram matmuls are emitted before the batch
        # setup so PE has real work while sq/sqn_row are still cooking.
        NPRE = 1
        pre = [emit_strip_pe(0, m) for m in range(NPRE)]
        batch_setup(0)
        for m in range(NPRE):
            emit_strip_offs(0, m, pre[m])
            emit_strip_drain(0, m, pre[m])
        for m in range(NPRE, MBLK):
            phs = emit_strip_pe(0, m)
            emit_strip_offs(0, m, phs)
            emit_strip_drain(0, m, phs)
        batch_setup(1)
        for m in range(MBLK):
            phs = emit_strip_pe(1, m)
            emit_strip_offs(1, m, phs)
            emit_strip_drain(1, m, phs)

        nc.sync.dma_start(rmax2_d.ap()[:, :, :], rmax2[:])
        nc.sync.dma_start(lsum2_d.ap()[:, :, :], lsum2[:])
        nc.sync.dma_start(sqn_d.ap()[:, :], sqn_sb[:])

    nc.compile()
    return nc


def _consts():
    neghalf = np.full((P, P), -0.5, dtype=ml_dtypes.bfloat16)
    ones32 = np.ones((P, 32), dtype=ml_dtypes.bfloat16)
    identneg = (NEGBIG * np.eye(P)).astype(ml_dtypes.bfloat16)
    ident = np.eye(P, dtype=np.float32).astype(ml_dtypes.bfloat16)
    return neghalf, ones32, identneg, ident


def make_in_maps(pred_poses):
    neghalf, ones32, identneg, ident = _consts()
    in_maps = []
    for c in range(NCORES):
        xb = pred_poses[c * BPC:(c + 1) * BPC]
        xt = np.ascontiguousarray(
            xb.transpose(0, 2, 1)).astype(ml_dtypes.float8_e4m3)
        in_maps.append({
            "xt": xt, "neghalf": neghalf, "ones32": ones32,
            "identneg": identneg, "ident": ident,
        })
    return in_maps


def postprocess(rmax2, lsum2, sqn):
    """[128,32,2],[128,32,2],[128,32] (one core) -> nnd2 [128,32] (f64)."""
    rmax2 = np.asarray(rmax2, dtype=np.float64)
    lsum2 = np.asarray(lsum2, dtype=np.float64)
    sqn = np.asarray(sqn, dtype=np.float64)
    nnd2 = np.empty((P, BPC * MBLK), dtype=np.float64)
    for col in range(BPC * MBLK):
        halves = []
        for h in range(2):
            if PAT2[col][h] == 'R':
                halves.append(sqn[:, col] - 2.0 * rmax2[:, col, h])
            else:
                halves.append(SOFT_C - (2.0 / SOFT_T) * np.log(
                    np.maximum(lsum2[:, col, h], 1e-300)))
        nnd2[:, col] = np.minimum(halves[0], halves[1])
    return np.maximum(nnd2, 0.0)


def kernel(pred_poses: np.ndarray) -> np.ndarray:
    pred_poses = np.ascontiguousarray(np.asarray(pred_poses, dtype=np.float32))
    assert pred_poses.shape == (B, N, D)

    if "nc" not in _CACHE:
        _CACHE["nc"] = build_kernel()
    nc = _CACHE["nc"]

    in_maps = make_in_maps(pred_poses)
    res = run_bass_kernel_spmd(nc, in_maps, list(range(NCORES)))

    nnd = np.zeros((B, N), dtype=np.float64)
    for c in range(NCORES):
        r = res.results[c]
        nnd2 = postprocess(r["rmax2"], r["lsum2"], r["sqn"])
        t = np.sqrt(nnd2)                               # [128, 32]
        for bl in range(BPC):
            sub = t[:, bl * MBLK:(bl + 1) * MBLK]       # [128, 16] (p, m)
            nnd[c * BPC + bl] = sub.T.reshape(N)        # index m*128+p

    mean = nnd.mean()
    std = nnd.std(ddof=1)
    eps = 1e-8
    cv = std / max(mean, eps) if mean > eps else 0.0
    return np.stack([mean, std, cv]).astype(np.float32)
